# revision 21
# baseline (speedup 1.0000x reference)
"""Trainium2 Bass kernel: single-query cross-attention (B=16, S=8192, D=1024).

Math trick: K and V are never materialized.
  scores[b,s] = (dec[b] @ W_Q.T @ W_K) . enc[b,s] / sqrt(D)
  context[b]  = (softmax(scores[b]) @ enc[b]) @ W_V.T
This reduces ~550 GFLOP of projections to ~0.6 GFLOP and makes the kernel
purely memory-bound: one streaming pass over encoder_outputs (512 MB).

Sharding: batch across the 8 cores (2 batches/core), no collectives.

Engine split (per core):
  DVE : scores (fused mul+row-reduce over enc), Q = W_Q @ dec, fin = W_V @ ctx0
  PE  : ctx0 += w . enc (w stationary, enc streams), qk = Q.T @ W_K,
        partition broadcasts, softmax-sum cross-partition reduce
  ACT : exp, PSUM->SBUF scaled copies
"""

import sys

if "/opt/trn_rl_repo" not in sys.path:
    sys.path.insert(0, "/opt/trn_rl_repo")

import numpy as np

import concourse.bass as bass
import concourse.tile as tile
from concourse import bacc, mybir
from concourse.masks import make_identity

B, S, D = 16, 8192, 1024
NCORES = 8
BL = B // NCORES  # batches per core
P = 128
DC = D // P  # d chunks of 128
FH = 512  # psum free-dim half (one bank of f32)
SCALE = 1.0 / 32.0  # 1/sqrt(D)
F32 = mybir.dt.float32
F32R = mybir.dt.float32r
MULT = mybir.AluOpType.mult


def build_program(s_len=S, grp=4, enc_bufs=4, mm_f32r=True):
    """Build the per-core SPMD Bass program."""
    nt = s_len // P  # number of 128-row s-tiles per batch
    ngrp = nt // grp  # enc DMA/exp groups

    nc = bacc.Bacc("TRN2", target_bir_lowering=False, debug=False, enable_asserts=False)
    ENC_DT = F32R if mm_f32r else F32
    enc = nc.dram_tensor("enc", [BL, s_len, D], ENC_DT, kind="ExternalInput")
    dec = nc.dram_tensor("dec", [BL, D], F32, kind="ExternalInput")
    wq = nc.dram_tensor("wq", [D, D], F32, kind="ExternalInput")
    wk = nc.dram_tensor("wk", [D, D], F32, kind="ExternalInput")
    wv = nc.dram_tensor("wv", [D, D], F32, kind="ExternalInput")
    ctx_out = nc.dram_tensor("ctx", [BL, D], F32, kind="ExternalOutput")
    attn_out = nc.dram_tensor("attn", [BL, s_len], F32, kind="ExternalOutput")

    with tile.TileContext(nc) as tc:
        with (
            tc.tile_pool(name="big", bufs=3) as big,
            tc.tile_pool(name="encp", bufs=enc_bufs) as encp,
            tc.tile_pool(name="small", bufs=1) as small,
            tc.tile_pool(name="stats", bufs=2) as stats,
            tc.tile_pool(name="pacc", bufs=2 * BL, space="PSUM") as pacc,
            tc.tile_pool(name="pscr", bufs=4, space="PSUM") as pscr,
        ):
            identity = small.tile([P, P], F32)
            make_identity(nc, identity)
            zbias = small.tile([P, 1], F32)
            nc.vector.memset(zbias, 0.0)
            ones_col = small.tile([P, 1], F32)
            nc.vector.memset(ones_col, 1.0)
            ones_row = small.tile([1, P], F32)
            nc.vector.memset(ones_row, 1.0)
            # per-batch selector rows: sel_b[x, :] = 1.0 iff x == b
            sels = []
            for b in range(BL):
                sel = small.tile([BL, P], F32, name=f"sel_{b}")
                nc.vector.memset(sel, 0.0)
                nc.gpsimd.affine_select(
                    out=sel,
                    in_=sel,
                    compare_op=mybir.AluOpType.not_equal,
                    fill=1.0,
                    base=-b,
                    pattern=[[0, P]],
                    channel_multiplier=1,
                )
                sels.append(sel)

            # ---- load weights: natural layout [e mod 128, e chunk, d] ----
            # dec + W_Q + W_K first (qk chain is the startup critical path),
            # 256KB chunks to spread across DMA queues. W_V is issued later.
            dec_sb = small.tile([BL, D], F32)
            nc.sync.dma_start(out=dec_sb, in_=dec[:, :])
            wq_sb = big.tile([P, DC, D], F32, tag="wbuf")
            for c in range(DC):
                for hh in range(2):
                    nc.sync.dma_start(
                        out=wq_sb[:, c, hh * FH : (hh + 1) * FH],
                        in_=wq[c * P : (c + 1) * P, hh * FH : (hh + 1) * FH],
                    )
            wk_sb = big.tile([P, DC, D], F32, tag="wbuf")
            for c in range(DC):
                for hh in range(2):
                    nc.sync.dma_start(
                        out=wk_sb[:, c, hh * FH : (hh + 1) * FH],
                        in_=wk[c * P : (c + 1) * P, hh * FH : (hh + 1) * FH],
                    )
            wv_sb = big.tile([P, DC, D], F32, tag="wbuf")

            # ---- dec broadcast to all partitions (per batch) via PE ----
            dec_rep = stats.tile([P, BL, D], F32, tag="rep4k", bufs=2, name="dec_rep")
            for b in range(BL):
                for h in range(2):
                    pb = pscr.tile([P, FH], F32, tag="ps", name=f"pdecb_{b}_{h}")
                    nc.tensor.matmul(
                        out=pb,
                        lhsT=sels[b][0:BL, :],
                        rhs=dec_sb[0:BL, h * FH : (h + 1) * FH],
                        start=True,
                        stop=True,
                    )
                    nc.scalar.copy(out=dec_rep[:, b, h * FH : (h + 1) * FH], in_=pb)

            # ---- Q^T[e, b] = sum_d W_Q[e, d] dec[b, d]  (DVE fused mul+reduce) ----
            scr0 = small.tile([P, D], F32)
            qT_sb = small.tile([P, DC, BL], F32)
            for b in range(BL):
                for c in range(DC):
                    nc.vector.scalar_tensor_tensor(
                        out=scr0,
                        in0=wq_sb[:, c, :],
                        scalar=1.0,
                        in1=dec_rep[:, b, :],
                        op0=MULT,
                        op1=MULT,
                        accum_out=qT_sb[:, c, b : b + 1],
                    )

            # ---- qk[b, d'] = sum_e Q[b, e] W_K[e, d']  (scaled by 1/sqrt(D)) ----
            qk_sb = small.tile([BL, D], F32)
            for h in range(2):
                pqk = pscr.tile([BL, FH], F32, tag="ps", name=f"pqk_{h}")
                for c in range(DC):
                    nc.tensor.matmul(
                        out=pqk,
                        lhsT=qT_sb[:, c, :],
                        rhs=wk_sb[:, c, h * FH : (h + 1) * FH],
                        start=(c == 0),
                        stop=(c == DC - 1),
                    )
                nc.scalar.activation(
                    out=qk_sb[0:BL, h * FH : (h + 1) * FH],
                    in_=pqk,
                    func=mybir.ActivationFunctionType.Copy,
                    bias=0.0,
                    scale=SCALE,
                )

            # ---- replicate qk rows to all 128 partitions (per batch) via PE ----
            qk_rep = small.tile([P, BL, D], F32)
            for b in range(BL):
                for h in range(2):
                    pb2 = pscr.tile([P, FH], F32, tag="ps", name=f"pqkb_{b}_{h}")
                    nc.tensor.matmul(
                        out=pb2,
                        lhsT=sels[b][0:BL, :],
                        rhs=qk_sb[0:BL, h * FH : (h + 1) * FH],
                        start=True,
                        stop=True,
                    )
                    nc.scalar.copy(out=qk_rep[:, b, h * FH : (h + 1) * FH], in_=pb2)

            # ---- main loop: stream enc; scores -> exp -> weighted-sum ----
            for b in range(BL):
                scores_b = stats.tile([P, nt], F32, tag="scores")
                w_b = stats.tile([P, nt], F32, tag="wexp")
                w_br = stats.tile([P, nt], ENC_DT, tag="wexpr", name="w_br") if mm_f32r else w_b
                scr = scr0
                acc = [
                    pacc.tile([1, FH], F32, tag="acc", name=f"acc_{b}_{h}")
                    for h in range(2)
                ]

                for g in range(ngrp):
                    if b == 0 and g == ngrp // 2:
                        # W_V arrives mid-stream: only needed by the epilogue
                        for c in range(DC):
                            for hh in range(2):
                                nc.sync.dma_start(
                                    out=wv_sb[:, c, hh * FH : (hh + 1) * FH],
                                    in_=wv[c * P : (c + 1) * P, hh * FH : (hh + 1) * FH],
                                )
                    enc_t = encp.tile([P, grp, D], ENC_DT, tag="enc")
                    for j in range(grp):
                        t = g * grp + j
                        nc.sync.dma_start(
                            out=enc_t[:, j, :], in_=enc[b, t * P : (t + 1) * P, :]
                        )
                    # scores: fused multiply + row-reduce on DVE
                    for j in range(grp):
                        t = g * grp + j
                        nc.vector.scalar_tensor_tensor(
                            out=scr,
                            in0=enc_t[:, j, :].bitcast(F32) if mm_f32r else enc_t[:, j, :],
                            scalar=1.0,
                            in1=qk_rep[:, b, :],
                            op0=MULT,
                            op1=MULT,
                            accum_out=scores_b[:, t : t + 1],
                        )
                    # exp for the whole group
                    nc.scalar.activation(
                        out=w_b[:, g * grp : (g + 1) * grp],
                        in_=scores_b[:, g * grp : (g + 1) * grp],
                        func=mybir.ActivationFunctionType.Exp,
                        bias=zbias,
                    )
                    if mm_f32r:
                        nc.scalar.copy(
                            out=w_br[:, g * grp : (g + 1) * grp],
                            in_=w_b[:, g * grp : (g + 1) * grp],
                        )
                    # ctx0 += w_t . enc_t  (w stationary, enc streams)
                    for j in range(grp):
                        t = g * grp + j
                        for h in range(2):
                            nc.tensor.matmul(
                                out=acc[h],
                                lhsT=w_br[:, t : t + 1],
                                rhs=enc_t[:, j, h * FH : (h + 1) * FH],
                                start=(t == 0),
                                stop=(t == nt - 1),
                            )

                # ---- epilogue for batch b ----
                # sumexp: row-reduce then cross-partition ones-matmul
                w_sum = stats.tile([P, 1], F32, tag="wsum")
                nc.vector.reduce_sum(out=w_sum, in_=w_b, axis=mybir.AxisListType.X)
                pse = pscr.tile([1, 1], F32, tag="ps", name=f"pse_{b}")
                nc.tensor.matmul(
                    out=pse, lhsT=ones_col, rhs=w_sum, start=True, stop=True
                )
                se_sb = stats.tile([1, 1], F32, tag="sesb")
                nc.vector.tensor_copy(out=se_sb, in_=pse)
                rinv1 = stats.tile([1, 1], F32, tag="rinv1")
                nc.vector.reciprocal(rinv1, se_sb)
                prb = pscr.tile([P, 1], F32, tag="ps", name=f"prb_{b}")
                nc.tensor.matmul(
                    out=prb, lhsT=ones_row, rhs=rinv1, start=True, stop=True
                )
                rinv = stats.tile([P, 1], F32, tag="rinv")
                nc.vector.tensor_copy(out=rinv, in_=prb)

                # attn weights: transpose [128, nt] -> [nt, 128], scale, store
                pat = pscr.tile([nt, P], F32, tag="ps", name=f"pat_{b}")
                nc.tensor.transpose(out=pat, in_=w_b, identity=identity)
                attn_sb = stats.tile([nt, P], F32, tag="attnsb")
                nc.vector.tensor_scalar_mul(
                    out=attn_sb, in0=pat, scalar1=rinv[0:nt, 0:1]
                )
                nc.sync.dma_start(
                    out=attn_out[b].rearrange("(t p) -> t p", p=P), in_=attn_sb
                )

                # ctx0 out of PSUM (unnormalized), broadcast to 128 partitions
                ctx_sb = stats.tile([1, D], F32, tag="ctxsb", bufs=1)
                for h in range(2):
                    nc.scalar.copy(
                        out=ctx_sb[0:1, h * FH : (h + 1) * FH], in_=acc[h]
                    )
                ctx_rep = stats.tile([P, D], F32, tag="rep4k", bufs=2, name="ctx_rep")
                for h in range(2):
                    pcb = pscr.tile([P, FH], F32, tag="ps", name=f"pcb_{b}_{h}")
                    nc.tensor.matmul(
                        out=pcb,
                        lhsT=ones_row,
                        rhs=ctx_sb[0:1, h * FH : (h + 1) * FH],
                        start=True,
                        stop=True,
                    )
                    nc.scalar.copy(out=ctx_rep[:, h * FH : (h + 1) * FH], in_=pcb)

                # fin^T[e] = sum_d W_V[e, d] ctx0[d]  (DVE fused mul+reduce)
                finT = stats.tile([P, DC], F32, tag="finT")
                for c in range(DC):
                    nc.vector.scalar_tensor_tensor(
                        out=scr,
                        in0=wv_sb[:, c, :],
                        scalar=1.0,
                        in1=ctx_rep,
                        op0=MULT,
                        op1=MULT,
                        accum_out=finT[:, c : c + 1],
                    )
                # normalize by 1/sumexp and store
                nc.vector.tensor_scalar_mul(out=finT, in0=finT, scalar1=rinv)
                nc.sync.dma_start(
                    out=ctx_out[b].rearrange("(c p) -> p c", p=P), in_=finT
                )

    nc.compile()
    return nc


_program_cache = {}


def _get_program(s_len=S, grp=4):
    key = (s_len, grp)
    if key not in _program_cache:
        _program_cache[key] = build_program(s_len, grp)
    return _program_cache[key]


def kernel(**inputs):
    from concourse.bass_utils import run_bass_kernel_spmd

    enc = np.ascontiguousarray(np.asarray(inputs["encoder_outputs"], np.float32))
    dec = np.ascontiguousarray(np.asarray(inputs["decoder_outputs"], np.float32))
    w_q = np.ascontiguousarray(np.asarray(inputs["W_Q"], np.float32))
    w_k = np.ascontiguousarray(np.asarray(inputs["W_K"], np.float32))
    w_v = np.ascontiguousarray(np.asarray(inputs["W_V"], np.float32))

    nc = _get_program()
    in_maps = []
    for c in range(NCORES):
        sl = slice(c * BL, (c + 1) * BL)
        in_maps.append(
            {"enc": enc[sl], "dec": dec[sl], "wq": w_q, "wk": w_k, "wv": w_v}
        )
    res = run_bass_kernel_spmd(nc, in_maps, list(range(NCORES)))
    ctx = np.concatenate([r["ctx"] for r in res.results], axis=0)
    attn = np.concatenate([r["attn"] for r in res.results], axis=0)[:, None, :]
    return ctx, attn


# revision 22
# speedup vs baseline: 1.1486x; 1.1486x over previous
"""Trainium2 Bass kernel: single-query cross-attention (B=16, S=8192, D=1024).

Math trick: K and V are never materialized.
  scores[b,s] = (dec[b] @ W_Q.T @ W_K) . enc[b,s] / sqrt(D)
  context[b]  = (softmax(scores[b]) @ enc[b]) @ W_V.T
This reduces ~550 GFLOP of projections to ~0.6 GFLOP and makes the kernel
purely memory-bound: one streaming pass over encoder_outputs (512 MB).

Sharding: batch across the 8 cores (2 batches/core), no collectives.

Engine split (per core):
  DVE : scores (fused mul+row-reduce over enc), Q = W_Q @ dec, fin = W_V @ ctx0
  PE  : ctx0 += w . enc (w stationary, enc streams), qk = Q.T @ W_K,
        partition broadcasts, softmax-sum cross-partition reduce
  ACT : exp, PSUM->SBUF scaled copies
"""

import sys

if "/opt/trn_rl_repo" not in sys.path:
    sys.path.insert(0, "/opt/trn_rl_repo")

import numpy as np

import concourse.bass as bass
import concourse.tile as tile
from concourse import bacc, mybir
from concourse.masks import make_identity

B, S, D = 16, 8192, 1024
NCORES = 8
BL = B // NCORES  # batches per core
P = 128
DC = D // P  # d chunks of 128
FH = 512  # psum free-dim half (one bank of f32)
SCALE = 1.0 / 32.0  # 1/sqrt(D)
F32 = mybir.dt.float32
F32R = mybir.dt.float32r
MULT = mybir.AluOpType.mult


def build_program(s_len=S, grp=4, enc_bufs=4, mm_f32r=True):
    """Build the per-core SPMD Bass program."""
    nt = s_len // P  # number of 128-row s-tiles per batch
    ngrp = nt // grp  # enc DMA/exp groups

    nc = bacc.Bacc("TRN2", target_bir_lowering=False, debug=False, enable_asserts=False)
    ENC_DT = F32R if mm_f32r else F32
    enc = nc.dram_tensor("enc", [BL, s_len, D], ENC_DT, kind="ExternalInput")
    dec = nc.dram_tensor("dec", [BL, D], F32, kind="ExternalInput")
    wq = nc.dram_tensor("wq", [D, D], F32, kind="ExternalInput")
    wk = nc.dram_tensor("wk", [D, D], F32, kind="ExternalInput")
    wv = nc.dram_tensor("wv", [D, D], F32, kind="ExternalInput")
    ctx_out = nc.dram_tensor("ctx", [BL, D], F32, kind="ExternalOutput")
    attn_out = nc.dram_tensor("attn", [BL, s_len], F32, kind="ExternalOutput")

    with tile.TileContext(nc) as tc:
        with (
            tc.tile_pool(name="big", bufs=3) as big,
            tc.tile_pool(name="encp", bufs=enc_bufs) as encp,
            tc.tile_pool(name="small", bufs=1) as small,
            tc.tile_pool(name="stats", bufs=2) as stats,
            tc.tile_pool(name="pacc", bufs=2 * BL, space="PSUM") as pacc,
            tc.tile_pool(name="pscr", bufs=4, space="PSUM") as pscr,
        ):
            identity = small.tile([P, P], F32)
            make_identity(nc, identity)
            zbias = small.tile([P, 1], F32)
            nc.vector.memset(zbias, 0.0)
            ones_col = small.tile([P, 1], F32)
            nc.vector.memset(ones_col, 1.0)
            ones_row = small.tile([1, P], F32)
            nc.vector.memset(ones_row, 1.0)
            # per-batch selector rows: sel_b[x, :] = 1.0 iff x == b
            sels = []
            for b in range(BL):
                sel = small.tile([BL, P], F32, name=f"sel_{b}")
                nc.vector.memset(sel, 0.0)
                nc.gpsimd.affine_select(
                    out=sel,
                    in_=sel,
                    compare_op=mybir.AluOpType.not_equal,
                    fill=1.0,
                    base=-b,
                    pattern=[[0, P]],
                    channel_multiplier=1,
                )
                sels.append(sel)

            # ---- load weights: natural layout [e mod 128, e chunk, d] ----
            # dec + W_Q + W_K first (qk chain is the startup critical path),
            # 256KB chunks to spread across DMA queues. W_V is issued later.
            dec_sb = small.tile([BL, D], F32)
            nc.sync.dma_start(out=dec_sb, in_=dec[:, :])
            wq_sb = big.tile([P, DC, D], F32, tag="wbuf")
            nc.sync.dma_start(
                out=wq_sb, in_=wq[:, :].rearrange("(c p) d -> p c d", p=P)
            )
            wk_sb = big.tile([P, DC, D], F32, tag="wbuf")
            nc.sync.dma_start(
                out=wk_sb, in_=wk[:, :].rearrange("(c p) d -> p c d", p=P)
            )
            wv_sb = big.tile([P, DC, D], F32, tag="wbuf")

            # ---- dec broadcast to all partitions (per batch) via PE ----
            dec_rep = stats.tile([P, BL, D], F32, tag="rep4k", bufs=2, name="dec_rep")
            for b in range(BL):
                for h in range(2):
                    pb = pscr.tile([P, FH], F32, tag="ps", name=f"pdecb_{b}_{h}")
                    nc.tensor.matmul(
                        out=pb,
                        lhsT=sels[b][0:BL, :],
                        rhs=dec_sb[0:BL, h * FH : (h + 1) * FH],
                        start=True,
                        stop=True,
                    )
                    nc.scalar.copy(out=dec_rep[:, b, h * FH : (h + 1) * FH], in_=pb)

            # ---- Q^T[e, b] = sum_d W_Q[e, d] dec[b, d]  (DVE fused mul+reduce) ----
            scr0 = small.tile([P, D], F32)
            qT_sb = small.tile([P, DC, BL], F32)
            for b in range(BL):
                for c in range(DC):
                    nc.vector.scalar_tensor_tensor(
                        out=scr0,
                        in0=wq_sb[:, c, :],
                        scalar=1.0,
                        in1=dec_rep[:, b, :],
                        op0=MULT,
                        op1=MULT,
                        accum_out=qT_sb[:, c, b : b + 1],
                    )

            # ---- qk[b, d'] = sum_e Q[b, e] W_K[e, d']  (scaled by 1/sqrt(D)) ----
            qk_sb = small.tile([BL, D], F32)
            for h in range(2):
                pqk = pscr.tile([BL, FH], F32, tag="ps", name=f"pqk_{h}")
                for c in range(DC):
                    nc.tensor.matmul(
                        out=pqk,
                        lhsT=qT_sb[:, c, :],
                        rhs=wk_sb[:, c, h * FH : (h + 1) * FH],
                        start=(c == 0),
                        stop=(c == DC - 1),
                    )
                nc.scalar.activation(
                    out=qk_sb[0:BL, h * FH : (h + 1) * FH],
                    in_=pqk,
                    func=mybir.ActivationFunctionType.Copy,
                    bias=0.0,
                    scale=SCALE,
                )

            # ---- replicate qk rows to all 128 partitions (per batch) via PE ----
            qk_rep = small.tile([P, BL, D], F32)
            for b in range(BL):
                for h in range(2):
                    pb2 = pscr.tile([P, FH], F32, tag="ps", name=f"pqkb_{b}_{h}")
                    nc.tensor.matmul(
                        out=pb2,
                        lhsT=sels[b][0:BL, :],
                        rhs=qk_sb[0:BL, h * FH : (h + 1) * FH],
                        start=True,
                        stop=True,
                    )
                    nc.scalar.copy(out=qk_rep[:, b, h * FH : (h + 1) * FH], in_=pb2)

            # ---- main loop: stream enc; scores -> exp -> weighted-sum ----
            for b in range(BL):
                scores_b = stats.tile([P, nt], F32, tag="scores")
                w_b = stats.tile([P, nt], F32, tag="wexp")
                w_br = stats.tile([P, nt], ENC_DT, tag="wexpr", name="w_br") if mm_f32r else w_b
                scr = scr0
                acc = [
                    pacc.tile([1, FH], F32, tag="acc", name=f"acc_{b}_{h}")
                    for h in range(2)
                ]

                for g in range(ngrp):
                    if b == 0 and g == ngrp // 2:
                        # W_V arrives mid-stream: only needed by the epilogue
                        nc.sync.dma_start(
                            out=wv_sb, in_=wv[:, :].rearrange("(c p) d -> p c d", p=P)
                        )
                    enc_t = encp.tile([P, grp, D], ENC_DT, tag="enc")
                    nc.sync.dma_start(
                        out=enc_t,
                        in_=enc[b, g * grp * P : (g + 1) * grp * P, :].rearrange(
                            "(t p) d -> p t d", p=P
                        ),
                    )
                    # scores: fused multiply + row-reduce on DVE
                    for j in range(grp):
                        t = g * grp + j
                        nc.vector.scalar_tensor_tensor(
                            out=scr,
                            in0=enc_t[:, j, :].bitcast(F32) if mm_f32r else enc_t[:, j, :],
                            scalar=1.0,
                            in1=qk_rep[:, b, :],
                            op0=MULT,
                            op1=MULT,
                            accum_out=scores_b[:, t : t + 1],
                        )
                    # exp for the whole group
                    nc.scalar.activation(
                        out=w_b[:, g * grp : (g + 1) * grp],
                        in_=scores_b[:, g * grp : (g + 1) * grp],
                        func=mybir.ActivationFunctionType.Exp,
                        bias=zbias,
                    )
                    if mm_f32r:
                        nc.scalar.copy(
                            out=w_br[:, g * grp : (g + 1) * grp],
                            in_=w_b[:, g * grp : (g + 1) * grp],
                        )
                    # ctx0 += w_t . enc_t  (w stationary, enc streams)
                    for j in range(grp):
                        t = g * grp + j
                        for h in range(2):
                            nc.tensor.matmul(
                                out=acc[h],
                                lhsT=w_br[:, t : t + 1],
                                rhs=enc_t[:, j, h * FH : (h + 1) * FH],
                                start=(t == 0),
                                stop=(t == nt - 1),
                            )

                # ---- epilogue for batch b ----
                # sumexp: row-reduce then cross-partition ones-matmul
                w_sum = stats.tile([P, 1], F32, tag="wsum")
                nc.vector.reduce_sum(out=w_sum, in_=w_b, axis=mybir.AxisListType.X)
                pse = pscr.tile([1, 1], F32, tag="ps", name=f"pse_{b}")
                nc.tensor.matmul(
                    out=pse, lhsT=ones_col, rhs=w_sum, start=True, stop=True
                )
                se_sb = stats.tile([1, 1], F32, tag="sesb")
                nc.vector.tensor_copy(out=se_sb, in_=pse)
                rinv1 = stats.tile([1, 1], F32, tag="rinv1")
                nc.vector.reciprocal(rinv1, se_sb)
                prb = pscr.tile([P, 1], F32, tag="ps", name=f"prb_{b}")
                nc.tensor.matmul(
                    out=prb, lhsT=ones_row, rhs=rinv1, start=True, stop=True
                )
                rinv = stats.tile([P, 1], F32, tag="rinv")
                nc.vector.tensor_copy(out=rinv, in_=prb)

                # attn weights: transpose [128, nt] -> [nt, 128], scale, store
                pat = pscr.tile([nt, P], F32, tag="ps", name=f"pat_{b}")
                nc.tensor.transpose(out=pat, in_=w_b, identity=identity)
                attn_sb = stats.tile([nt, P], F32, tag="attnsb")
                nc.vector.tensor_scalar_mul(
                    out=attn_sb, in0=pat, scalar1=rinv[0:nt, 0:1]
                )
                nc.sync.dma_start(
                    out=attn_out[b].rearrange("(t p) -> t p", p=P), in_=attn_sb
                )

                # ctx0 out of PSUM (unnormalized), broadcast to 128 partitions
                ctx_sb = stats.tile([1, D], F32, tag="ctxsb", bufs=1)
                for h in range(2):
                    nc.scalar.copy(
                        out=ctx_sb[0:1, h * FH : (h + 1) * FH], in_=acc[h]
                    )
                ctx_rep = stats.tile([P, D], F32, tag="rep4k", bufs=2, name="ctx_rep")
                for h in range(2):
                    pcb = pscr.tile([P, FH], F32, tag="ps", name=f"pcb_{b}_{h}")
                    nc.tensor.matmul(
                        out=pcb,
                        lhsT=ones_row,
                        rhs=ctx_sb[0:1, h * FH : (h + 1) * FH],
                        start=True,
                        stop=True,
                    )
                    nc.scalar.copy(out=ctx_rep[:, h * FH : (h + 1) * FH], in_=pcb)

                # fin^T[e] = sum_d W_V[e, d] ctx0[d]  (DVE fused mul+reduce)
                finT = stats.tile([P, DC], F32, tag="finT")
                for c in range(DC):
                    nc.vector.scalar_tensor_tensor(
                        out=scr,
                        in0=wv_sb[:, c, :],
                        scalar=1.0,
                        in1=ctx_rep,
                        op0=MULT,
                        op1=MULT,
                        accum_out=finT[:, c : c + 1],
                    )
                # normalize by 1/sumexp and store
                nc.vector.tensor_scalar_mul(out=finT, in0=finT, scalar1=rinv)
                nc.sync.dma_start(
                    out=ctx_out[b].rearrange("(c p) -> p c", p=P), in_=finT
                )

    nc.compile()
    return nc


_program_cache = {}


def _get_program(s_len=S, grp=4):
    key = (s_len, grp)
    if key not in _program_cache:
        _program_cache[key] = build_program(s_len, grp)
    return _program_cache[key]


def kernel(**inputs):
    from concourse.bass_utils import run_bass_kernel_spmd

    enc = np.ascontiguousarray(np.asarray(inputs["encoder_outputs"], np.float32))
    dec = np.ascontiguousarray(np.asarray(inputs["decoder_outputs"], np.float32))
    w_q = np.ascontiguousarray(np.asarray(inputs["W_Q"], np.float32))
    w_k = np.ascontiguousarray(np.asarray(inputs["W_K"], np.float32))
    w_v = np.ascontiguousarray(np.asarray(inputs["W_V"], np.float32))

    nc = _get_program()
    in_maps = []
    for c in range(NCORES):
        sl = slice(c * BL, (c + 1) * BL)
        in_maps.append(
            {"enc": enc[sl], "dec": dec[sl], "wq": w_q, "wk": w_k, "wv": w_v}
        )
    res = run_bass_kernel_spmd(nc, in_maps, list(range(NCORES)))
    ctx = np.concatenate([r["ctx"] for r in res.results], axis=0)
    attn = np.concatenate([r["attn"] for r in res.results], axis=0)[:, None, :]
    return ctx, attn
